# revision 6
# baseline (speedup 1.0000x reference)
"""Trainium2 Bass kernel v3 for nn_BasicConv_78915729097031 (e3nn GNN conv).

Math per edge e (i=src, j=dst):
    w_e   = radial_mlp(emb_e)                # [4096] per-edge TP weights
    msg_e = TP(x[i_e], sh_e, w_e)            # [128]
    out[n] = (1/sqrt(8)) * sum_{e: j_e=n} msg_e

v3 design: the v2 kernel was DVE-bound (97% busy) on six 1-elem/cycle
fused multiply+cumsum scans per 128-edge tile (6144 products/partition).
v3 halves most of that by running the scan in the DVE's 2x_1p packed
mode via a hand-authored uops_2x program (fp16 pairs in both ports,
p0/p1 multiplied on stages 0-1, pair-summed, accumulated at stage 3
with a 1-cycle feedback, s_lo/s_hi packed into wr0_lo/hi):
  * quarters a, b, d: ACT converts wq PSUM fp32 -> SBUF fp16 (one copy
    per quarter; d's single copy feeds 3 scans), then 2x scans write
    dense fp16 cumsum streams into one scratch tile laid out
    [a|d0|d1|d2|c'0|c'1|c'2|b] x 1024;
  * quarter c stays a 1x scan straight from PSUM (boundary-broadcast
    out, as v2) -> its 32 bounds are expanded x sh1_k by ACT into the
    c' scratch slots (strided writes at the page-boundary positions);
  * the scatter matmuls read the page-boundary columns of the scratch
    directly via stride-32 APs (no separate extraction), one fp16
    [128,224] matmul + one [128,32] b-matmul per tile, sharing one
    LDWEIGHTS, accumulating in PSUM per chunk with exactly ONE
    start=True group (second start=True in a bank drops the first
    group's has_written accumulation);
  * per-w sums are recovered by differencing page-boundary cumsums on
    the host AFTER the linear scatter (cumsum and segment-sum commute).
This balances DVE (5 x 2x scans + 1 x 1x scan) against ACT (3 converts
+ silu + c'), with PE (MLP1/MLP2/scatter) and DMA well under both.
"""
import os
import sys

import numpy as np

for _p in ("/opt/trn_rl_repo", "/root/.axon_site/_ro/trn_rl_repo"):
    if os.path.isdir(_p) and _p not in sys.path:
        sys.path.insert(0, _p)
        break

MUL = 32
N_NODES = 8192
N_EDGES = 65536
INV_SQRT3 = 1.0 / np.sqrt(3.0)
NORM0 = np.sqrt(1.0 / (2.0 * MUL))
NORM1 = np.sqrt(3.0 / (2.0 * MUL))
SILU_GAIN = 1.6790
NUM_NEIGHBORS = 8.0
NC = 8
NPC = 128
CHUNKS_PER_CORE = (N_NODES // NPC) // NC   # 8

# av column layout (fp16): [a(0:32) | b(32:64) | d0 d1 d2 (64:160) |
#                           c(160:192) | sh1 (192:195)]
AV_A, AV_B, AV_D, AV_C, AV_SH1 = 0, 32, 64, 160, 192
AV_COLS = 195

# scratch (scr) slot layout, 1024 fp16 cols each; the scatter reads the
# page-boundary positions 31::32:
#   [a | d0 | d1 | d2 | c'0 | c'1 | c'2 | b]
SCR_A, SCR_D, SCR_CP, SCR_B, SCR_C = 0, 1024, 4096, 7168, 8192
SCR_COLS = 8192  # +1024 when the c-quarter also runs as a 2x scan

# scatter PSUM col layout: [out0 (0:32) | d-part (32:128) | c'-part (128:224)]
M_COLS = 224


# --------------------------------------------------------------------------- #
# Host-side preparation
# --------------------------------------------------------------------------- #
def _balance_nodes(j):
    """Relabel nodes so each 128-node chunk has <= 1024 incoming edges.

    Greedy LPT bin-packing by in-degree + a swap repair pass. Returns
    node_perm with node_perm[new_id] = original node id.
    """
    deg = np.bincount(j, minlength=N_NODES).astype(np.int64)
    nbins = N_NODES // NPC
    cap = NPC * 8
    bin_e = np.zeros(nbins, np.int64)
    bin_n = np.zeros(nbins, np.int64)
    members = [[] for _ in range(nbins)]
    big = np.int64(1 << 60)
    for nd in np.argsort(-deg, kind="stable"):
        be = np.where(bin_n < NPC, bin_e, big)
        b = int(be.argmin())
        members[b].append(int(nd))
        bin_e[b] += deg[nd]
        bin_n[b] += 1
    for _ in range(300):
        over = np.where(bin_e > cap)[0]
        if not len(over):
            break
        progressed = False
        for ob in over:
            x = int(bin_e[ob] - cap)
            if x <= 0:
                continue
            degs_ob = {}
            for a in members[ob]:
                degs_ob.setdefault(int(deg[a]), a)
            done = False
            for ub in np.argsort(bin_e):
                y = int(cap - bin_e[ub])
                if y <= 0:
                    continue
                hi = min(x, y)
                degs_ub = {}
                for a in members[ub]:
                    degs_ub.setdefault(int(deg[a]), a)
                for delta in range(hi, 0, -1):
                    for db, bnode in degs_ub.items():
                        da = db + delta
                        if da in degs_ob:
                            anode = degs_ob[da]
                            members[ob].remove(anode)
                            members[ub].remove(bnode)
                            members[ob].append(bnode)
                            members[ub].append(anode)
                            bin_e[ob] -= delta
                            bin_e[ub] += delta
                            done = True
                            progressed = True
                            break
                    if done:
                        break
                if done:
                    break
        if not progressed:
            break
    node_perm = np.concatenate([np.array(m, np.int64) for m in members])
    return node_perm


def _host_prep(x, edge_index, edge_attr, edge_len_emb, W1, W2):
    i = edge_index[0].astype(np.int64)
    j_orig = edge_index[1].astype(np.int64)
    E = i.shape[0]
    node_perm = _balance_nodes(j_orig)
    newid = np.empty(N_NODES, np.int64)
    newid[node_perm] = np.arange(N_NODES)
    j = newid[j_orig]
    order = np.argsort(j, kind="stable")
    i_s, j_s = i[order], j[order]
    sh = edge_attr[order].astype(np.float32)
    emb = edge_len_emb[order].astype(np.float32)
    xg = x[i_s].astype(np.float32)
    x0 = xg[:, :MUL]
    x1 = xg[:, MUL:].reshape(E, MUL, 3)
    sh0 = sh[:, 0]
    sh1 = sh[:, 1:4]

    s8 = 1.0 / np.sqrt(NUM_NEIGHBORS)
    av = np.zeros((E, AV_COLS), np.float32)
    av[:, AV_A:AV_A + 32] = x0 * sh0[:, None] * (NORM0 * s8)
    av[:, AV_B:AV_B + 32] = (np.einsum("eui,ei->eu", x1, sh1)
                             * (INV_SQRT3 * NORM0 * s8))
    for k in range(3):
        av[:, AV_D + 32 * k:AV_D + 32 * k + 32] = (
            x1[:, :, k] * (sh0[:, None] * (INV_SQRT3 * NORM1 * s8)))
    av[:, AV_C:AV_C + 32] = x0 * (INV_SQRT3 * NORM1 * s8)
    av[:, AV_SH1:AV_SH1 + 3] = sh1

    W1eff = (W1 / np.sqrt(W1.shape[0])).astype(np.float32)              # [64,128]
    W2eff = (SILU_GAIN * W2 / np.sqrt(W2.shape[0])).astype(np.float32)  # [128,4096]
    # quarter q = path block [a,b,c,d]; within a quarter (w outer, u inner)
    W2eff = (W2eff.reshape(128, 4, MUL, MUL)      # [h, path, u, w]
             .transpose(0, 1, 3, 2)               # [h, path, w, u]
             .reshape(128, 4096).copy())

    n_chunks = N_NODES // NPC
    chunk_of_edge = j_s // NPC
    counts = np.bincount(chunk_of_edge, minlength=n_chunks)
    tiles_of_chunk = np.maximum(1, np.ceil(counts / 128).astype(np.int64))

    order2 = np.argsort(-tiles_of_chunk, kind="stable")
    assign = np.empty((NC, CHUNKS_PER_CORE), np.int64)
    for s in range(CHUNKS_PER_CORE):
        row = order2[s * NC:(s + 1) * NC]
        assign[:, s] = row if s % 2 == 0 else row[::-1]
    schedule = tuple(int(tiles_of_chunk[assign[:, s]].max())
                     for s in range(CHUNKS_PER_CORE))
    slot_base = np.concatenate([[0], np.cumsum(np.array(schedule) * 128)])
    e_pad = int(slot_base[-1])
    n_tiles = sum(schedule)

    import ml_dtypes
    bf16 = ml_dtypes.bfloat16
    embT = np.zeros((NC, n_tiles, 64, 128), bf16)
    avx = np.zeros((NC, e_pad, AV_COLS), np.float16)
    sh1f = np.zeros((NC, e_pad, 3), np.float32)
    s16 = np.zeros((NC, n_tiles, 128, 128), np.float16)
    starts = np.concatenate([[0], np.cumsum(counts)])
    eye = np.eye(128, dtype=np.float32)
    for core in range(NC):
        for s in range(CHUNKS_PER_CORE):
            c = int(assign[core, s])
            lo, hi = int(starts[c]), int(starts[c + 1])
            cnt = hi - lo
            base = int(slot_base[s])
            et = emb[lo:hi].T                      # [64, cnt]
            etp = np.zeros((64, schedule[s] * 128), np.float32)
            etp[:, :cnt] = et
            t0 = sum(schedule[:s])
            embT[core, t0:t0 + schedule[s]] = (
                etp.reshape(64, schedule[s], 128).transpose(1, 0, 2)
                .astype(bf16))
            avx[core, base:base + cnt] = av[lo:hi].astype(np.float16)
            sh1f[core, base:base + cnt] = sh1[lo:hi]
            nloc = (j_s[lo:hi] - c * NPC).astype(np.int64)
            onehot = np.zeros((schedule[s] * 128, 128), np.float32)
            onehot[np.arange(cnt)] = eye[nloc]
            s16[core, t0:t0 + schedule[s]] = (
                onehot.reshape(schedule[s], 128, 128).astype(np.float16))
    return dict(embT=embT, avx=avx, sh1f=sh1f, s16=s16,
                W1eff=W1eff.astype(bf16), W2eff=W2eff.astype(np.float16),
                schedule=schedule, e_pad=e_pad, n_tiles=n_tiles,
                assign=(assign, node_perm))


# --------------------------------------------------------------------------- #
# Custom DVE op: fused multiply + running cumsum, with a hand-authored
# 2x_1p uop program (2 fp16 elems/cycle when all operands are dense fp16).
# --------------------------------------------------------------------------- #
_SCAN_OP_NAME = "TT_MUL_CUMSUM2X_ANT"


def _register_scan_op():
    import concourse.dve_ops as dve_ops
    for o in dve_ops.OPS:
        if o.name == _SCAN_OP_NAME:
            return o
    from concourse.dve_spec import Spec, Src0, Src1, scan, AluOp, lower, _has_src1
    from concourse.dve_uop import (
        DveOpSpec, UopConfig, UopDpConfig, InpSel, AluInp, DelayInp, OutSel,
        OutPath, Trigger, ENABLE, DISABLE,
    )

    def _ref(in0, in1, s0, s1, imm2):
        prod = in0.astype(np.float32) * in1.astype(np.float32)
        flat = prod.reshape(prod.shape[0], -1)
        return np.cumsum(flat, axis=-1).reshape(prod.shape)

    spec = Spec(body=scan(AluOp.ADD, Src0 * Src1), reference=_ref)
    uops_1x = lower(spec, ver="v3")
    assert len(uops_1x) == 2

    # 2x_1p program, 2 states mirroring the 1x FSM (setup seeds the
    # stage-3 accumulator flop from the hard-wired zero lane; steady
    # computes p0/p1 from the lo/hi packed halves, pair-sums, and scans).
    def base_uop():
        u = UopConfig()
        u.enable_input(InpSel.SRC_0, 1)       # delay_0 = in0 lo
        u.enable_input(InpSel.SRC_1, 2)       # delay_1 = in1 lo
        u.enable_input(InpSel.SRC_0_HI, 3)    # delay_2 = in0 hi
        u.enable_input(InpSel.SRC_1_HI, 4)    # delay_3 = in1 hi
        u.enable_input(InpSel.ZERO, 5)        # delay_4 = 0 (acc seed)
        u.datapath_config[0] = (
            UopDpConfig()
            .enable_alu(AluOp.MULTIPLY, AluInp.PREV_DELAY_0,
                        AluInp.PREV_DELAY_1)
            .pass_through_delay(2, 3, 4))
        u.datapath_config[1] = (
            UopDpConfig()
            .enable_alu(AluOp.MULTIPLY, AluInp.PREV_DELAY_2,
                        AluInp.PREV_DELAY_3)
            .enable_delay_from_src(DelayInp.PREV_ALU_OUT, 0)
            .pass_through_delay(4))
        u.datapath_config[2] = (
            UopDpConfig()
            .enable_alu(AluOp.ADD, AluInp.PREV_ALU_OUT, AluInp.PREV_DELAY_0)
            .enable_delay_from_src(DelayInp.PREV_ALU_OUT, 1)
            .pass_through_delay(4))
        u.datapath_config[3] = (
            UopDpConfig()
            .enable_alu(AluOp.ADD, AluInp.CURR_ALU_OUT, AluInp.PREV_ALU_OUT)
            .pass_through_delay(1))
        u.datapath_config[4] = (
            UopDpConfig()
            .enable_alu(AluOp.SUBTRACT, AluInp.PREV_ALU_OUT,
                        AluInp.PREV_DELAY_1)
            .enable_delay_from_src(DelayInp.PREV_ALU_OUT, 2))
        for s in (5, 6, 7):
            u.datapath_config[s] = (
                UopDpConfig().pass_through_alu().pass_through_delay(2))
        return u

    setup = base_uop()
    setup.datapath_config[3] = (
        UopDpConfig()
        .enable_alu(AluOp.BYPASS, AluInp.PREV_DELAY_4, AluInp.PREV_DELAY_4)
        .pass_through_delay(1))
    setup.require_inp0 = DISABLE
    setup.require_inp1 = DISABLE
    setup.repeat_count = 1
    setup.trigger = (Trigger.COUNT, Trigger.NONE, Trigger.NONE)
    setup.next_uop = (1, 0, 0)

    steady = base_uop()
    steady.require_inp0 = ENABLE
    steady.require_inp1 = ENABLE
    steady.trigger = (Trigger.SRC_TENSOR_DONE, Trigger.NONE, Trigger.NONE)
    steady.next_uop = (0, 0, 0)
    steady.enable_output(OutSel.ALU_OUT, OutPath.WR0_LO)
    steady.enable_output(OutSel.DELAY_2, OutPath.WR0_HI)

    opcode = dve_ops._CUSTOM_DVE_ROW_BASE + len(dve_ops.OPS)
    spec_full = DveOpSpec(
        name=_SCAN_OP_NAME, opcode=opcode, uops=uops_1x,
        uops_2x=[setup, steady], perf_max=1, rd1_en=_has_src1(spec))
    shas = {ver: spec_full.sha(ver) for ver in ("v3", "v4")}
    op = dve_ops.DveOp(_SCAN_OP_NAME, spec, subdim=True, uops_sha=shas)
    dve_ops.OPS.append(op)
    dve_ops._SUB_OPCODE_FOR_NAME[_SCAN_OP_NAME] = opcode
    dve_ops.CUSTOM_DVE_SPECS[_SCAN_OP_NAME] = spec
    dve_ops._COMPILE_CACHE[(_SCAN_OP_NAME, "v3")] = spec_full
    dve_ops._COMPILE_CACHE[(_SCAN_OP_NAME, "v4")] = spec_full
    return op


def _emit_scan(vec, op, *, out, in0, in1, perf_max):
    """nc.vector._custom_dve clone that sets perf_max at construction
    (mutating the returned wrapper does not reach the stored instruction)."""
    import concourse.bass_isa as bass_isa
    from concourse import mybir
    from concourse.dve_ops import get_dve_sub_opcode
    from concourse.dve_table_gen import dve_ver_for

    bass = vec.bass
    if op.name not in bass.m.ant_custom_dve_ops:
        bass.m.ant_custom_dve_ops = sorted(
            {*bass.m.ant_custom_dve_ops, op.name})
    compiled = op.compile(dve_ver_for(bass.trn_type))
    opt = not op.subdim
    shape = (bass_isa.CustomDveShape.STT if len(in1.shape) > 2
             else bass_isa.CustomDveShape.TTSS)
    isa_opcode = bass.isa.Opcode[
        f"NEURON_ISA_TPB_OPCODE_CUSTOM_DVE_ANT_{shape.slot()}"].value
    ins = [vec.lower_ap(in0, for_isa=True, opt=opt),
           vec.lower_ap(in1, for_isa=True, opt=opt),
           mybir.ImmediateValue(dtype=mybir.dt.float32, value=0.0),
           mybir.ImmediateValue(dtype=mybir.dt.float32, value=0.0)]
    outs = [vec.lower_ap(out, for_isa=True, opt=opt)]
    return vec.add_instruction(
        bass_isa.InstCustomDveAnt(
            name=bass.get_next_instruction_name(),
            op_name=op.name,
            rd1_en=compiled.rd1_en,
            subdim=0x02 if op.subdim else 0,
            imm2=0.0,
            shape=shape,
            row=get_dve_sub_opcode(op.name),
            isa_opcode=isa_opcode,
            perf_max=perf_max,
            ins=ins,
            outs=outs,
        ))


# --------------------------------------------------------------------------- #
# Bass program
# --------------------------------------------------------------------------- #
_PROGRAM_CACHE = {}

# scheduling knobs (sim-tuned): PSUM pool depths and the c'-expansion engine
_WPS_BUFS = int(os.environ.get("KV3_WPS", "2"))
_MPS_BUFS = int(os.environ.get("KV3_MPS", "2"))
_CP_ENGINE = os.environ.get("KV3_CP", "gps")   # act | gps
_C2X = os.environ.get("KV3_C2X", "0") == "1"   # c-quarter: 2x scan via 4th convert
_PSDMA = os.environ.get("KV3_PSDMA", "0") == "1"  # DMA chunk out straight from PSUM


def _build_program(schedule, e_pad, repeat=1):
    key = (schedule, e_pad, repeat, _WPS_BUFS, _MPS_BUFS, _CP_ENGINE, _C2X, _PSDMA)
    if key in _PROGRAM_CACHE:
        return _PROGRAM_CACHE[key]

    from concourse import bacc, mybir
    import concourse.tile as tile

    scan_op = _register_scan_op()

    f32 = mybir.dt.float32
    f16 = mybir.dt.float16
    bf16 = mybir.dt.bfloat16
    AF = mybir.ActivationFunctionType

    nc = bacc.Bacc("TRN2", target_bir_lowering=False, debug=False,
                   num_devices=NC)

    n_tiles = sum(schedule)
    embT_d = nc.dram_tensor("embT", [n_tiles, 64, 128], bf16,
                            kind="ExternalInput").ap()
    avx_d = nc.dram_tensor("avx", [e_pad, AV_COLS], f16,
                           kind="ExternalInput").ap()
    s_d = nc.dram_tensor("s16", [n_tiles, 128, 128], f16,
                         kind="ExternalInput").ap()
    sh1_d = nc.dram_tensor("sh1f", [e_pad, 3], f32,
                           kind="ExternalInput").ap()
    w1_d = nc.dram_tensor("w1", [64, 128], bf16, kind="ExternalInput").ap()
    w2_d = nc.dram_tensor("w2", [128, 4096], f16, kind="ExternalInput").ap()
    out_d = nc.dram_tensor("out", [CHUNKS_PER_CORE * 128, M_COLS], f32,
                           kind="ExternalOutput").ap()

    with tile.TileContext(nc) as tc:
        with (
            tc.tile_pool(name="const", bufs=1) as const_p,
            tc.tile_pool(name="inp", bufs=6) as inp_p,
            tc.tile_pool(name="hsb", bufs=4) as h_p,
            tc.tile_pool(name="cnv", bufs=4) as cnv_p,
            tc.tile_pool(name="scr", bufs=2) as scr_p,
            tc.tile_pool(name="rdc", bufs=2) as rdc_p,
            tc.tile_pool(name="osb", bufs=2) as out_p,
            tc.tile_pool(name="hps", bufs=1, space="PSUM") as hps_p,
            tc.tile_pool(name="wps", bufs=_WPS_BUFS, space="PSUM") as wps_p,
            tc.tile_pool(name="mps", bufs=_MPS_BUFS, space="PSUM") as mps_p,
        ):
            w1_sb = const_p.tile([64, 128], bf16)
            nc.sync.dma_start(w1_sb[:], w1_d[:])
            w2_sb = const_p.tile([128, 4096], f16)
            nc.sync.dma_start(w2_sb[:], w2_d[:])

            def scan2x(cnv, av_sb, av_off, scr, scr_off):
                _emit_scan(
                    nc.vector, scan_op,
                    out=scr[:, scr_off:scr_off + 1024]
                        .rearrange("p (w u) -> p w u", u=32),
                    in0=cnv[:].rearrange("p (w u) -> p w u", u=32),
                    in1=av_sb[:, av_off:av_off + 32]
                        .rearrange("p u -> p () u")
                        .to_broadcast([128, 32, 32]),
                    perf_max=1)

            for cc_rep in range(CHUNKS_PER_CORE * repeat):
                cc = cc_rep % CHUNKS_PER_CORE
                m_ps = mps_p.tile([128, M_COLS], f32, space="PSUM", tag="m")
                pending = None
                tpc = schedule[cc]
                t_base = sum(schedule[:cc])
                for t in range(tpc):
                    til = t_base + t
                    e0 = til * 128
                    first, last = t == 0, t == tpc - 1

                    # ---- loads ----
                    emb_sb = inp_p.tile([64, 128], bf16, tag="emb")
                    nc.sync.dma_start(emb_sb[:], embT_d[til])
                    av_sb = inp_p.tile([128, AV_COLS], f16, tag="av")
                    nc.sync.dma_start(av_sb[:], avx_d[e0:e0 + 128, :])
                    s_sb = inp_p.tile([128, 128], f16, tag="s")
                    nc.sync.dma_start(s_sb[:], s_d[til])
                    sh1_sb = inp_p.tile([128, 3], f32, tag="sh1")
                    nc.sync.dma_start(sh1_sb[:], sh1_d[e0:e0 + 128, :])

                    # ---- MLP1 + silu -> h [128h, 128e] fp16 ----
                    hpre = hps_p.tile([128, 128], f32, space="PSUM",
                                      tag="hpre")
                    nc.tensor.matmul(hpre[:], lhsT=w1_sb[:], rhs=emb_sb[:],
                                     start=True, stop=True)
                    h_sb = h_p.tile([128, 128], f16, tag="h")
                    nc.scalar.activation(h_sb[:], hpre[:], AF.Silu)

                    scr = scr_p.tile(
                        [128, SCR_COLS + (1024 if _C2X else 0)], f16,
                        tag="scr")
                    red_c = (None if _C2X else
                             rdc_p.tile([128, 32], f32, tag="rc"))

                    # quarter order: c first (its consumer is the 1x DVE
                    # scan straight from PSUM -- no ACT convert), then
                    # d/a/b whose converts pipeline on ACT.
                    for q, qn in ((2, "c"), (3, "d"), (0, "a"), (1, "b")):
                        wq_ps = wps_p.tile([128, 1024], f32, space="PSUM",
                                           tag="wq")
                        for half in range(2):
                            sl = slice(half * 512, half * 512 + 512)
                            nc.tensor.matmul(
                                wq_ps[:, sl], lhsT=h_sb[:],
                                rhs=w2_sb[:, q * 1024 + half * 512:
                                          q * 1024 + half * 512 + 512],
                                start=True, stop=True)
                        if qn == "c" and not _C2X:
                            # 1x scan from PSUM; boundary-broadcast out
                            _emit_scan(
                                nc.vector, scan_op,
                                out=red_c[:]
                                    .rearrange("p w -> p w ()")
                                    .to_broadcast([128, 32, 32]),
                                in0=wq_ps[:].rearrange("p (w u) -> p w u",
                                                       u=32),
                                in1=av_sb[:, AV_C:AV_C + 32]
                                    .rearrange("p u -> p () u")
                                    .to_broadcast([128, 32, 32]),
                                perf_max=0)
                        elif qn == "c":
                            cnv = cnv_p.tile([128, 1024], f16,
                                             tag="cc", name="cv_c")
                            nc.scalar.copy(out=cnv[:], in_=wq_ps[:])
                            scan2x(cnv, av_sb, AV_C, scr, SCR_C)
                        else:
                            cnv = cnv_p.tile([128, 1024], f16,
                                             tag="c" + qn, name="cv_" + qn)
                            nc.scalar.copy(out=cnv[:], in_=wq_ps[:])
                            if qn == "d":
                                for k in range(3):
                                    scan2x(cnv, av_sb, AV_D + 32 * k,
                                           scr, SCR_D + 1024 * k)
                            elif qn == "a":
                                scan2x(cnv, av_sb, AV_A, scr, SCR_A)
                            else:
                                scan2x(cnv, av_sb, AV_B, scr, SCR_B)

                    # ---- c' = c-bounds * sh1_k into scratch boundary
                    # slots (strided fp16 writes) ----
                    for k in range(3):
                        cp_out = (scr[:, SCR_CP + 1024 * k:
                                      SCR_CP + 1024 * (k + 1)]
                                  .rearrange("p (w u) -> p w u", u=32)
                                  [:, :, 31:32])
                        if _C2X:
                            cp_in = (scr[:, SCR_C:SCR_C + 1024]
                                     .rearrange("p (w u) -> p w u", u=32)
                                     [:, :, 31:32])
                        else:
                            cp_in = red_c[:].rearrange("p w -> p w ()")
                        if _CP_ENGINE == "gps":
                            nc.gpsimd.tensor_scalar_mul(
                                cp_out, cp_in, sh1_sb[:, k:k + 1])
                        else:
                            nc.scalar.activation(
                                cp_out, cp_in, AF.Copy,
                                scale=sh1_sb[:, k:k + 1])

                    # ---- scatter, delayed one tile: issuing tile t-1's
                    # scatter after tile t's MLP2 keeps the PE FIFO from
                    # blocking on this tile's scans (chunk accumulation
                    # commutes; still exactly ONE start=True per chunk) ----
                    def emit_scatter(ps_sb, pscr, pfirst, plast):
                        bounds = (pscr[:]
                                  .rearrange("p (c u) -> p c u", u=32)
                                  [:, :, 31:32])
                        nc.tensor.matmul(
                            m_ps[:, 0:224], lhsT=ps_sb[:],
                            rhs=bounds[:, 0:224], start=pfirst, stop=False,
                            skip_group_check=True)
                        nc.tensor.matmul(
                            m_ps[:, 0:32], lhsT=ps_sb[:],
                            rhs=bounds[:, 224:256],
                            start=False, stop=plast,
                            skip_group_check=True)

                    if pending is not None:
                        emit_scatter(*pending)
                    pending = (s_sb, scr, first, last)

                if pending is not None:
                    emit_scatter(*pending)
                    pending = None

                # ---- store chunk ----
                if _PSDMA:
                    nc.sync.dma_start(out_d[cc * 128:(cc + 1) * 128, :],
                                      m_ps[:])
                else:
                    o_sb = out_p.tile([128, M_COLS], f32, tag="o")
                    nc.scalar.copy(out=o_sb[:], in_=m_ps[:])
                    nc.sync.dma_start(out_d[cc * 128:(cc + 1) * 128, :],
                                      o_sb[:])

    nc.compile()
    _PROGRAM_CACHE[key] = nc
    return nc


# --------------------------------------------------------------------------- #
# Entry point
# --------------------------------------------------------------------------- #
def _build_in_maps(prep):
    in_maps = []
    for c in range(NC):
        in_maps.append({
            "embT": prep["embT"][c],
            "avx": prep["avx"][c],
            "s16": prep["s16"][c],
            "sh1f": prep["sh1f"][c],
            "w1": prep["W1eff"],
            "w2": prep["W2eff"],
        })
    return in_maps


def _postprocess(per_core_out, assign, plan_s=None):
    assign, node_perm = assign
    M = np.empty((N_NODES, M_COLS), np.float32)
    for core in range(NC):
        for s in range(CHUNKS_PER_CORE):
            c = int(assign[core, s])
            M[c * NPC:(c + 1) * NPC] = per_core_out[core][s * NPC:(s + 1) * NPC]

    def blkdiff(B):
        # per-32-block de-cumsum along columns
        out = B.copy()
        out[:, 1:] -= B[:, :-1]
        return out

    out0 = blkdiff(M[:, 0:32])        # cum(a)+cum(b) both reset at col 0
    dpart = M[:, 32:128].reshape(N_NODES, 3, 32)
    cpart = M[:, 128:224].reshape(N_NODES, 3, 32)
    dpart = np.concatenate(
        [blkdiff(dpart[:, k])[:, None, :] for k in range(3)], axis=1)
    cpart = np.concatenate(
        [blkdiff(cpart[:, k])[:, None, :] for k in range(3)], axis=1)
    out1 = dpart + cpart              # [N, 3, 32] (k, w)
    out_rel = np.empty((N_NODES, 128), np.float32)
    out_rel[:, :32] = out0
    out_rel[:, 32:] = out1.transpose(0, 2, 1).reshape(N_NODES, 96)  # (w, k)
    out = np.empty_like(out_rel)
    out[node_perm] = out_rel
    return out


def _prepare(x, edge_index, edge_attr, edge_len_emb, W1, W2, repeat=1,
             plan_s=None):
    x = np.asarray(x, np.float32)
    edge_index = np.asarray(edge_index)
    edge_attr = np.asarray(edge_attr, np.float32)
    edge_len_emb = np.asarray(edge_len_emb, np.float32)
    W1 = np.asarray(W1, np.float32)
    W2 = np.asarray(W2, np.float32)
    prep = _host_prep(x, edge_index, edge_attr, edge_len_emb, W1, W2)
    nc = _build_program(prep["schedule"], prep["e_pad"], repeat=repeat)
    return prep, nc, _build_in_maps(prep)


def kernel(x, edge_index, edge_attr, edge_len_emb, W1, W2, _results_out=None):
    prep, nc, in_maps = _prepare(x, edge_index, edge_attr, edge_len_emb,
                                 W1, W2)

    from concourse.bass_utils import run_bass_kernel_spmd

    res = run_bass_kernel_spmd(nc, in_maps, core_ids=list(range(NC)))
    if _results_out is not None:
        _results_out.append(res)

    return _postprocess([res.results[c]["out"] for c in range(NC)],
                        prep["assign"])


# revision 9
# speedup vs baseline: 1.4754x; 1.4754x over previous
"""Trainium2 Bass kernel v3 for nn_BasicConv_78915729097031 (e3nn GNN conv).

Math per edge e (i=src, j=dst):
    w_e   = radial_mlp(emb_e)                # [4096] per-edge TP weights
    msg_e = TP(x[i_e], sh_e, w_e)            # [128]
    out[n] = (1/sqrt(8)) * sum_{e: j_e=n} msg_e

v3 design: the v2 kernel was DVE-bound (97% busy) on six 1-elem/cycle
fused multiply+cumsum scans per 128-edge tile (6144 products/partition).
v3 halves most of that by running the scan in the DVE's 2x_1p packed
mode via a hand-authored uops_2x program (fp16 pairs in both ports,
p0/p1 multiplied on stages 0-1, pair-summed, accumulated at stage 3
with a 1-cycle feedback, s_lo/s_hi packed into wr0_lo/hi):
  * quarters a, b, d: ACT converts wq PSUM fp32 -> SBUF fp16 (one copy
    per quarter; d's single copy feeds 3 scans), then 2x scans write
    dense fp16 cumsum streams into one scratch tile laid out
    [a|d0|d1|d2|c'0|c'1|c'2|b] x 1024;
  * quarter c stays a 1x scan straight from PSUM (boundary-broadcast
    out, as v2) -> its 32 bounds are expanded x sh1_k by ACT into the
    c' scratch slots (strided writes at the page-boundary positions);
  * the scatter matmuls read the page-boundary columns of the scratch
    directly via stride-32 APs (no separate extraction), one fp16
    [128,224] matmul + one [128,32] b-matmul per tile, sharing one
    LDWEIGHTS, accumulating in PSUM per chunk with exactly ONE
    start=True group (second start=True in a bank drops the first
    group's has_written accumulation);
  * per-w sums are recovered by differencing page-boundary cumsums on
    the host AFTER the linear scatter (cumsum and segment-sum commute).
This balances DVE (5 x 2x scans + 1 x 1x scan) against ACT (3 converts
+ silu + c'), with PE (MLP1/MLP2/scatter) and DMA well under both.
"""
import os
import sys

import numpy as np

for _p in ("/opt/trn_rl_repo", "/root/.axon_site/_ro/trn_rl_repo"):
    if os.path.isdir(_p) and _p not in sys.path:
        sys.path.insert(0, _p)
        break

MUL = 32
N_NODES = 8192
N_EDGES = 65536
INV_SQRT3 = 1.0 / np.sqrt(3.0)
NORM0 = np.sqrt(1.0 / (2.0 * MUL))
NORM1 = np.sqrt(3.0 / (2.0 * MUL))
SILU_GAIN = 1.6790
NUM_NEIGHBORS = 8.0
NC = 8
NPC = 128
CHUNKS_PER_CORE = (N_NODES // NPC) // NC   # 8

# av column layout (fp16): [a(0:32) | b(32:64) | d0 d1 d2 (64:160) |
#                           c(160:192) | sh1 (192:195)]
AV_A, AV_B, AV_D, AV_C, AV_SH1 = 0, 32, 64, 160, 192
AV_COLS = 195
# combined DMA row: [s one-hot (0:128) | av (128:323)]
SAV_AV = 128
SAV_COLS = 128 + AV_COLS

# scratch (scr) slot layout, 1024 fp16 cols each; the scatter reads the
# page-boundary positions 31::32:
#   [a | d0 | d1 | d2 | c'0 | c'1 | c'2 | b]
SCR_A, SCR_D, SCR_CP, SCR_B, SCR_C = 0, 1024, 4096, 7168, 8192
SCR_COLS = 8192  # +1024 when the c-quarter also runs as a 2x scan

# scatter PSUM col layout: [out0 (0:32) | d-part (32:128) | c'-part (128:224)]
M_COLS = 224


# --------------------------------------------------------------------------- #
# Host-side preparation
# --------------------------------------------------------------------------- #
def _balance_nodes(j):
    """Relabel nodes so each 128-node chunk has <= 1024 incoming edges.

    Greedy LPT bin-packing by in-degree + a swap repair pass. Returns
    node_perm with node_perm[new_id] = original node id.
    """
    deg = np.bincount(j, minlength=N_NODES).astype(np.int64)
    nbins = N_NODES // NPC
    cap = NPC * 8
    bin_e = np.zeros(nbins, np.int64)
    bin_n = np.zeros(nbins, np.int64)
    members = [[] for _ in range(nbins)]
    big = np.int64(1 << 60)
    for nd in np.argsort(-deg, kind="stable"):
        be = np.where(bin_n < NPC, bin_e, big)
        b = int(be.argmin())
        members[b].append(int(nd))
        bin_e[b] += deg[nd]
        bin_n[b] += 1
    for _ in range(300):
        over = np.where(bin_e > cap)[0]
        if not len(over):
            break
        progressed = False
        for ob in over:
            x = int(bin_e[ob] - cap)
            if x <= 0:
                continue
            degs_ob = {}
            for a in members[ob]:
                degs_ob.setdefault(int(deg[a]), a)
            done = False
            for ub in np.argsort(bin_e):
                y = int(cap - bin_e[ub])
                if y <= 0:
                    continue
                hi = min(x, y)
                degs_ub = {}
                for a in members[ub]:
                    degs_ub.setdefault(int(deg[a]), a)
                for delta in range(hi, 0, -1):
                    for db, bnode in degs_ub.items():
                        da = db + delta
                        if da in degs_ob:
                            anode = degs_ob[da]
                            members[ob].remove(anode)
                            members[ub].remove(bnode)
                            members[ob].append(bnode)
                            members[ub].append(anode)
                            bin_e[ob] -= delta
                            bin_e[ub] += delta
                            done = True
                            progressed = True
                            break
                    if done:
                        break
                if done:
                    break
        if not progressed:
            break
    node_perm = np.concatenate([np.array(m, np.int64) for m in members])
    return node_perm


def _host_prep(x, edge_index, edge_attr, edge_len_emb, W1, W2):
    i = edge_index[0].astype(np.int64)
    j_orig = edge_index[1].astype(np.int64)
    E = i.shape[0]
    node_perm = _balance_nodes(j_orig)
    newid = np.empty(N_NODES, np.int64)
    newid[node_perm] = np.arange(N_NODES)
    j = newid[j_orig]
    order = np.argsort(j, kind="stable")
    i_s, j_s = i[order], j[order]
    sh = edge_attr[order].astype(np.float32)
    emb = edge_len_emb[order].astype(np.float32)
    xg = x[i_s].astype(np.float32)
    x0 = xg[:, :MUL]
    x1 = xg[:, MUL:].reshape(E, MUL, 3)
    sh0 = sh[:, 0]
    sh1 = sh[:, 1:4]

    s8 = 1.0 / np.sqrt(NUM_NEIGHBORS)
    av = np.zeros((E, AV_COLS), np.float32)
    av[:, AV_A:AV_A + 32] = x0 * sh0[:, None] * (NORM0 * s8)
    av[:, AV_B:AV_B + 32] = (np.einsum("eui,ei->eu", x1, sh1)
                             * (INV_SQRT3 * NORM0 * s8))
    for k in range(3):
        av[:, AV_D + 32 * k:AV_D + 32 * k + 32] = (
            x1[:, :, k] * (sh0[:, None] * (INV_SQRT3 * NORM1 * s8)))
    av[:, AV_C:AV_C + 32] = x0 * (INV_SQRT3 * NORM1 * s8)
    av[:, AV_SH1:AV_SH1 + 3] = sh1

    W1eff = (W1 / np.sqrt(W1.shape[0])).astype(np.float32)              # [64,128]
    W2eff = (SILU_GAIN * W2 / np.sqrt(W2.shape[0])).astype(np.float32)  # [128,4096]
    # quarter q = path block [a,b,c,d]; within a quarter (w outer, u inner)
    W2eff = (W2eff.reshape(128, 4, MUL, MUL)      # [h, path, u, w]
             .transpose(0, 1, 3, 2)               # [h, path, w, u]
             .reshape(128, 4096).copy())

    n_chunks = N_NODES // NPC
    chunk_of_edge = j_s // NPC
    counts = np.bincount(chunk_of_edge, minlength=n_chunks)
    tiles_of_chunk = np.maximum(1, np.ceil(counts / 128).astype(np.int64))

    order2 = np.argsort(-tiles_of_chunk, kind="stable")
    assign = np.empty((NC, CHUNKS_PER_CORE), np.int64)
    for s in range(CHUNKS_PER_CORE):
        row = order2[s * NC:(s + 1) * NC]
        assign[:, s] = row if s % 2 == 0 else row[::-1]
    schedule = tuple(int(tiles_of_chunk[assign[:, s]].max())
                     for s in range(CHUNKS_PER_CORE))
    slot_base = np.concatenate([[0], np.cumsum(np.array(schedule) * 128)])
    e_pad = int(slot_base[-1])
    n_tiles = sum(schedule)

    import ml_dtypes
    bf16 = ml_dtypes.bfloat16
    embT = np.zeros((NC, n_tiles, 64, 128), bf16)
    # combined per-edge-row tensor: [one-hot scatter col (128) | av (195)]
    sav = np.zeros((NC, e_pad, SAV_COLS), np.float16)
    starts = np.concatenate([[0], np.cumsum(counts)])
    eye = np.eye(128, dtype=np.float32)
    for core in range(NC):
        for s in range(CHUNKS_PER_CORE):
            c = int(assign[core, s])
            lo, hi = int(starts[c]), int(starts[c + 1])
            cnt = hi - lo
            base = int(slot_base[s])
            et = emb[lo:hi].T                      # [64, cnt]
            etp = np.zeros((64, schedule[s] * 128), np.float32)
            etp[:, :cnt] = et
            t0 = sum(schedule[:s])
            embT[core, t0:t0 + schedule[s]] = (
                etp.reshape(64, schedule[s], 128).transpose(1, 0, 2)
                .astype(bf16))
            sav[core, base:base + cnt, SAV_AV:] = av[lo:hi].astype(np.float16)
            nloc = (j_s[lo:hi] - c * NPC).astype(np.int64)
            sav[core, np.arange(base, base + cnt), nloc] = 1.0
    return dict(embT=embT, sav=sav,
                W1eff=W1eff.astype(bf16), W2eff=W2eff.astype(np.float16),
                schedule=schedule, e_pad=e_pad, n_tiles=n_tiles,
                assign=(assign, node_perm))


# --------------------------------------------------------------------------- #
# Custom DVE op: fused multiply + running cumsum, with a hand-authored
# 2x_1p uop program (2 fp16 elems/cycle when all operands are dense fp16).
# --------------------------------------------------------------------------- #
_SCAN_OP_NAME = "TT_MUL_CUMSUM2X_ANT"


def _register_scan_op():
    import concourse.dve_ops as dve_ops
    for o in dve_ops.OPS:
        if o.name == _SCAN_OP_NAME:
            return o
    from concourse.dve_spec import Spec, Src0, Src1, scan, AluOp, lower, _has_src1
    from concourse.dve_uop import (
        DveOpSpec, UopConfig, UopDpConfig, InpSel, AluInp, DelayInp, OutSel,
        OutPath, Trigger, ENABLE, DISABLE,
    )

    def _ref(in0, in1, s0, s1, imm2):
        prod = in0.astype(np.float32) * in1.astype(np.float32)
        flat = prod.reshape(prod.shape[0], -1)
        return np.cumsum(flat, axis=-1).reshape(prod.shape)

    spec = Spec(body=scan(AluOp.ADD, Src0 * Src1), reference=_ref)
    uops_1x = lower(spec, ver="v3")
    assert len(uops_1x) == 2

    # 2x_1p program, 2 states mirroring the 1x FSM (setup seeds the
    # stage-3 accumulator flop from the hard-wired zero lane; steady
    # computes p0/p1 from the lo/hi packed halves, pair-sums, and scans).
    def base_uop():
        u = UopConfig()
        u.enable_input(InpSel.SRC_0, 1)       # delay_0 = in0 lo
        u.enable_input(InpSel.SRC_1, 2)       # delay_1 = in1 lo
        u.enable_input(InpSel.SRC_0_HI, 3)    # delay_2 = in0 hi
        u.enable_input(InpSel.SRC_1_HI, 4)    # delay_3 = in1 hi
        u.enable_input(InpSel.ZERO, 5)        # delay_4 = 0 (acc seed)
        u.datapath_config[0] = (
            UopDpConfig()
            .enable_alu(AluOp.MULTIPLY, AluInp.PREV_DELAY_0,
                        AluInp.PREV_DELAY_1)
            .pass_through_delay(2, 3, 4))
        u.datapath_config[1] = (
            UopDpConfig()
            .enable_alu(AluOp.MULTIPLY, AluInp.PREV_DELAY_2,
                        AluInp.PREV_DELAY_3)
            .enable_delay_from_src(DelayInp.PREV_ALU_OUT, 0)
            .pass_through_delay(4))
        u.datapath_config[2] = (
            UopDpConfig()
            .enable_alu(AluOp.ADD, AluInp.PREV_ALU_OUT, AluInp.PREV_DELAY_0)
            .enable_delay_from_src(DelayInp.PREV_ALU_OUT, 1)
            .pass_through_delay(4))
        u.datapath_config[3] = (
            UopDpConfig()
            .enable_alu(AluOp.ADD, AluInp.CURR_ALU_OUT, AluInp.PREV_ALU_OUT)
            .pass_through_delay(1))
        u.datapath_config[4] = (
            UopDpConfig()
            .enable_alu(AluOp.SUBTRACT, AluInp.PREV_ALU_OUT,
                        AluInp.PREV_DELAY_1)
            .enable_delay_from_src(DelayInp.PREV_ALU_OUT, 2))
        for s in (5, 6, 7):
            u.datapath_config[s] = (
                UopDpConfig().pass_through_alu().pass_through_delay(2))
        return u

    setup = base_uop()
    setup.datapath_config[3] = (
        UopDpConfig()
        .enable_alu(AluOp.BYPASS, AluInp.PREV_DELAY_4, AluInp.PREV_DELAY_4)
        .pass_through_delay(1))
    setup.require_inp0 = DISABLE
    setup.require_inp1 = DISABLE
    setup.repeat_count = 1
    setup.trigger = (Trigger.COUNT, Trigger.NONE, Trigger.NONE)
    setup.next_uop = (1, 0, 0)

    steady = base_uop()
    steady.require_inp0 = ENABLE
    steady.require_inp1 = ENABLE
    steady.trigger = (Trigger.SRC_TENSOR_DONE, Trigger.NONE, Trigger.NONE)
    steady.next_uop = (0, 0, 0)
    steady.enable_output(OutSel.ALU_OUT, OutPath.WR0_LO)
    steady.enable_output(OutSel.DELAY_2, OutPath.WR0_HI)

    opcode = dve_ops._CUSTOM_DVE_ROW_BASE + len(dve_ops.OPS)
    spec_full = DveOpSpec(
        name=_SCAN_OP_NAME, opcode=opcode, uops=uops_1x,
        uops_2x=[setup, steady], perf_max=1, rd1_en=_has_src1(spec))
    shas = {ver: spec_full.sha(ver) for ver in ("v3", "v4")}
    op = dve_ops.DveOp(_SCAN_OP_NAME, spec, subdim=True, uops_sha=shas)
    dve_ops.OPS.append(op)
    dve_ops._SUB_OPCODE_FOR_NAME[_SCAN_OP_NAME] = opcode
    dve_ops.CUSTOM_DVE_SPECS[_SCAN_OP_NAME] = spec
    dve_ops._COMPILE_CACHE[(_SCAN_OP_NAME, "v3")] = spec_full
    dve_ops._COMPILE_CACHE[(_SCAN_OP_NAME, "v4")] = spec_full
    return op


def _emit_scan(vec, op, *, out, in0, in1, perf_max):
    """nc.vector._custom_dve clone that sets perf_max at construction
    (mutating the returned wrapper does not reach the stored instruction)."""
    import concourse.bass_isa as bass_isa
    from concourse import mybir
    from concourse.dve_ops import get_dve_sub_opcode
    from concourse.dve_table_gen import dve_ver_for

    bass = vec.bass
    if op.name not in bass.m.ant_custom_dve_ops:
        bass.m.ant_custom_dve_ops = sorted(
            {*bass.m.ant_custom_dve_ops, op.name})
    compiled = op.compile(dve_ver_for(bass.trn_type))
    opt = not op.subdim
    shape = (bass_isa.CustomDveShape.STT if len(in1.shape) > 2
             else bass_isa.CustomDveShape.TTSS)
    isa_opcode = bass.isa.Opcode[
        f"NEURON_ISA_TPB_OPCODE_CUSTOM_DVE_ANT_{shape.slot()}"].value
    ins = [vec.lower_ap(in0, for_isa=True, opt=opt),
           vec.lower_ap(in1, for_isa=True, opt=opt),
           mybir.ImmediateValue(dtype=mybir.dt.float32, value=0.0),
           mybir.ImmediateValue(dtype=mybir.dt.float32, value=0.0)]
    outs = [vec.lower_ap(out, for_isa=True, opt=opt)]
    return vec.add_instruction(
        bass_isa.InstCustomDveAnt(
            name=bass.get_next_instruction_name(),
            op_name=op.name,
            rd1_en=compiled.rd1_en,
            subdim=0x02 if op.subdim else 0,
            imm2=0.0,
            shape=shape,
            row=get_dve_sub_opcode(op.name),
            isa_opcode=isa_opcode,
            perf_max=perf_max,
            ins=ins,
            outs=outs,
        ))


# --------------------------------------------------------------------------- #
# Bass program
# --------------------------------------------------------------------------- #
_PROGRAM_CACHE = {}


def _knobs():
    """Scheduling knobs (read per build so A/B harnesses can vary them):
    PSUM pool depths, c'-expansion engine, c-quarter 2x mode."""
    return (int(os.environ.get("KV3_WPS", "2")),
            int(os.environ.get("KV3_MPS", "2")),
            os.environ.get("KV3_CP", "gps"),
            os.environ.get("KV3_C2X", "0") == "1",
            os.environ.get("KV3_PSDMA", "0") == "1")


def _build_program(schedule, e_pad, repeat=1):
    _WPS_BUFS, _MPS_BUFS, _CP_ENGINE, _C2X, _PSDMA = _knobs()
    key = (schedule, e_pad, repeat, _WPS_BUFS, _MPS_BUFS, _CP_ENGINE, _C2X, _PSDMA)
    if key in _PROGRAM_CACHE:
        return _PROGRAM_CACHE[key]

    from concourse import bacc, mybir
    import concourse.tile as tile

    scan_op = _register_scan_op()

    f32 = mybir.dt.float32
    f16 = mybir.dt.float16
    bf16 = mybir.dt.bfloat16
    AF = mybir.ActivationFunctionType
    OP = mybir.AluOpType

    nc = bacc.Bacc("TRN2", target_bir_lowering=False, debug=False,
                   num_devices=NC)

    n_tiles = sum(schedule)
    embT_d = nc.dram_tensor("embT", [n_tiles, 64, 128], bf16,
                            kind="ExternalInput").ap()
    sav_d = nc.dram_tensor("sav", [e_pad, SAV_COLS], f16,
                           kind="ExternalInput").ap()
    w1_d = nc.dram_tensor("w1", [64, 128], bf16, kind="ExternalInput").ap()
    w2_d = nc.dram_tensor("w2", [128, 4096], f16, kind="ExternalInput").ap()
    out_d = nc.dram_tensor("out", [CHUNKS_PER_CORE * 128, M_COLS], f32,
                           kind="ExternalOutput").ap()

    with tile.TileContext(nc) as tc:
        with (
            tc.tile_pool(name="const", bufs=1) as const_p,
            tc.tile_pool(name="inp", bufs=6) as inp_p,
            tc.tile_pool(name="hsb", bufs=4) as h_p,
            tc.tile_pool(name="cnv", bufs=4) as cnv_p,
            tc.tile_pool(name="scr", bufs=2) as scr_p,
            tc.tile_pool(name="rdc", bufs=2) as rdc_p,
            tc.tile_pool(name="osb", bufs=2) as out_p,
            tc.tile_pool(name="hps", bufs=1, space="PSUM") as hps_p,
            tc.tile_pool(name="wps", bufs=_WPS_BUFS, space="PSUM") as wps_p,
            tc.tile_pool(name="mps", bufs=_MPS_BUFS, space="PSUM") as mps_p,
        ):
            w1_sb = const_p.tile([64, 128], bf16)
            nc.sync.dma_start(w1_sb[:], w1_d[:])
            w2_sb = const_p.tile([128, 4096], f16)
            nc.sync.dma_start(w2_sb[:], w2_d[:])

            def scan2x(cnv, av_sb, av_off, scr, scr_off):
                _emit_scan(
                    nc.vector, scan_op,
                    out=scr[:, scr_off:scr_off + 1024]
                        .rearrange("p (w u) -> p w u", u=32),
                    in0=cnv[:].rearrange("p (w u) -> p w u", u=32),
                    in1=av_sb[:, av_off:av_off + 32]
                        .rearrange("p u -> p () u")
                        .to_broadcast([128, 32, 32]),
                    perf_max=1)

            for cc_rep in range(CHUNKS_PER_CORE * repeat):
                cc = cc_rep % CHUNKS_PER_CORE
                m_ps = mps_p.tile([128, M_COLS], f32, space="PSUM", tag="m")
                pending = None
                tpc = schedule[cc]
                t_base = sum(schedule[:cc])
                for t in range(tpc):
                    til = t_base + t
                    e0 = til * 128
                    first, last = t == 0, t == tpc - 1

                    # ---- loads ----
                    emb_sb = inp_p.tile([64, 128], bf16, tag="emb")
                    nc.sync.dma_start(emb_sb[:], embT_d[til])
                    sav_sb = inp_p.tile([128, SAV_COLS], f16, tag="sav")
                    nc.sync.dma_start(sav_sb[:], sav_d[e0:e0 + 128, :])
                    s_sb = sav_sb[:, 0:128]
                    av_sb = sav_sb[:, SAV_AV:]

                    # ---- MLP1 + silu -> h [128h, 128e] fp16 ----
                    hpre = hps_p.tile([128, 128], f32, space="PSUM",
                                      tag="hpre")
                    nc.tensor.matmul(hpre[:], lhsT=w1_sb[:], rhs=emb_sb[:],
                                     start=True, stop=True)
                    h_sb = h_p.tile([128, 128], f16, tag="h")
                    nc.scalar.activation(h_sb[:], hpre[:], AF.Silu)

                    scr = scr_p.tile(
                        [128, SCR_COLS + (1024 if _C2X else 0)], f16,
                        tag="scr")
                    red_c = (None if _C2X else
                             rdc_p.tile([128, 32], f32, tag="rc"))

                    # quarter order: c first (its consumer is the 1x DVE
                    # scan straight from PSUM -- no ACT convert), then
                    # d/a/b whose converts pipeline on ACT.
                    for q, qn in ((2, "c"), (3, "d"), (0, "a"), (1, "b")):
                        wq_ps = wps_p.tile([128, 1024], f32, space="PSUM",
                                           tag="wq")
                        for half in range(2):
                            sl = slice(half * 512, half * 512 + 512)
                            nc.tensor.matmul(
                                wq_ps[:, sl], lhsT=h_sb[:],
                                rhs=w2_sb[:, q * 1024 + half * 512:
                                          q * 1024 + half * 512 + 512],
                                start=True, stop=True)
                        if qn == "c" and not _C2X:
                            # 1x scan from PSUM; boundary-broadcast out
                            _emit_scan(
                                nc.vector, scan_op,
                                out=red_c[:]
                                    .rearrange("p w -> p w ()")
                                    .to_broadcast([128, 32, 32]),
                                in0=wq_ps[:].rearrange("p (w u) -> p w u",
                                                       u=32),
                                in1=av_sb[:, AV_C:AV_C + 32]
                                    .rearrange("p u -> p () u")
                                    .to_broadcast([128, 32, 32]),
                                perf_max=0)
                        elif qn == "c":
                            cnv = cnv_p.tile([128, 1024], f16,
                                             tag="cc", name="cv_c")
                            nc.scalar.copy(out=cnv[:], in_=wq_ps[:])
                            scan2x(cnv, av_sb, AV_C, scr, SCR_C)
                        else:
                            cnv = cnv_p.tile([128, 1024], f16,
                                             tag="c" + qn, name="cv_" + qn)
                            nc.scalar.copy(out=cnv[:], in_=wq_ps[:])
                            if qn == "d":
                                for k in range(3):
                                    scan2x(cnv, av_sb, AV_D + 32 * k,
                                           scr, SCR_D + 1024 * k)
                            elif qn == "a":
                                scan2x(cnv, av_sb, AV_A, scr, SCR_A)
                            else:
                                scan2x(cnv, av_sb, AV_B, scr, SCR_B)

                    # ---- c' = c-bounds * sh1_k into the scratch boundary
                    # slots, all 3 k at once (strided fp16 writes) ----
                    cp_out = (scr[:, SCR_CP:SCR_CP + 3072]
                              .rearrange("p (k w u) -> p (k w) u", k=3, u=32)
                              [:, :, 31:32]
                              .rearrange("p (k w) o -> p k (w o)", k=3))
                    if _C2X:
                        cp_in = (scr[:, SCR_C:SCR_C + 1024]
                                 .rearrange("p (w u) -> p w u", u=32)
                                 [:, :, 31:32]
                                 .rearrange("p w o -> p (o w)"))
                    else:
                        cp_in = red_c[:]
                    eng = nc.gpsimd if _CP_ENGINE == "gps" else nc.vector
                    eng.tensor_tensor(
                        out=cp_out,
                        in0=cp_in.rearrange("p w -> p () w")
                            .to_broadcast([128, 3, 32]),
                        in1=av_sb[:, AV_SH1:AV_SH1 + 3]
                            .rearrange("p k -> p k ()")
                            .to_broadcast([128, 3, 32]),
                        op=OP.mult)

                    # ---- scatter, delayed one tile: issuing tile t-1's
                    # scatter after tile t's MLP2 keeps the PE FIFO from
                    # blocking on this tile's scans (chunk accumulation
                    # commutes; still exactly ONE start=True per chunk) ----
                    def emit_scatter(ps_sb, pscr, pfirst, plast):
                        bounds = (pscr[:]
                                  .rearrange("p (c u) -> p c u", u=32)
                                  [:, :, 31:32])
                        nc.tensor.matmul(
                            m_ps[:, 0:224], lhsT=ps_sb[:],
                            rhs=bounds[:, 0:224], start=pfirst, stop=False,
                            skip_group_check=True)
                        nc.tensor.matmul(
                            m_ps[:, 0:32], lhsT=ps_sb[:],
                            rhs=bounds[:, 224:256],
                            start=False, stop=plast,
                            skip_group_check=True)

                    if pending is not None:
                        emit_scatter(*pending)
                    pending = (s_sb, scr, first, last)

                if pending is not None:
                    emit_scatter(*pending)
                    pending = None

                # ---- store chunk ----
                if _PSDMA:
                    nc.sync.dma_start(out_d[cc * 128:(cc + 1) * 128, :],
                                      m_ps[:])
                else:
                    o_sb = out_p.tile([128, M_COLS], f32, tag="o")
                    nc.scalar.copy(out=o_sb[:], in_=m_ps[:])
                    nc.sync.dma_start(out_d[cc * 128:(cc + 1) * 128, :],
                                      o_sb[:])

    nc.compile()
    _PROGRAM_CACHE[key] = nc
    return nc


# --------------------------------------------------------------------------- #
# Entry point
# --------------------------------------------------------------------------- #
def _build_in_maps(prep):
    in_maps = []
    for c in range(NC):
        in_maps.append({
            "embT": prep["embT"][c],
            "sav": prep["sav"][c],
            "w1": prep["W1eff"],
            "w2": prep["W2eff"],
        })
    return in_maps


def _postprocess(per_core_out, assign, plan_s=None):
    assign, node_perm = assign
    M = np.empty((N_NODES, M_COLS), np.float32)
    for core in range(NC):
        for s in range(CHUNKS_PER_CORE):
            c = int(assign[core, s])
            M[c * NPC:(c + 1) * NPC] = per_core_out[core][s * NPC:(s + 1) * NPC]

    def blkdiff(B):
        # per-32-block de-cumsum along columns
        out = B.copy()
        out[:, 1:] -= B[:, :-1]
        return out

    out0 = blkdiff(M[:, 0:32])        # cum(a)+cum(b) both reset at col 0
    dpart = M[:, 32:128].reshape(N_NODES, 3, 32)
    cpart = M[:, 128:224].reshape(N_NODES, 3, 32)
    dpart = np.concatenate(
        [blkdiff(dpart[:, k])[:, None, :] for k in range(3)], axis=1)
    cpart = np.concatenate(
        [blkdiff(cpart[:, k])[:, None, :] for k in range(3)], axis=1)
    out1 = dpart + cpart              # [N, 3, 32] (k, w)
    out_rel = np.empty((N_NODES, 128), np.float32)
    out_rel[:, :32] = out0
    out_rel[:, 32:] = out1.transpose(0, 2, 1).reshape(N_NODES, 96)  # (w, k)
    out = np.empty_like(out_rel)
    out[node_perm] = out_rel
    return out


def _prepare(x, edge_index, edge_attr, edge_len_emb, W1, W2, repeat=1,
             plan_s=None):
    x = np.asarray(x, np.float32)
    edge_index = np.asarray(edge_index)
    edge_attr = np.asarray(edge_attr, np.float32)
    edge_len_emb = np.asarray(edge_len_emb, np.float32)
    W1 = np.asarray(W1, np.float32)
    W2 = np.asarray(W2, np.float32)
    prep = _host_prep(x, edge_index, edge_attr, edge_len_emb, W1, W2)
    nc = _build_program(prep["schedule"], prep["e_pad"], repeat=repeat)
    return prep, nc, _build_in_maps(prep)


def kernel(x, edge_index, edge_attr, edge_len_emb, W1, W2, _results_out=None):
    prep, nc, in_maps = _prepare(x, edge_index, edge_attr, edge_len_emb,
                                 W1, W2)

    from concourse.bass_utils import run_bass_kernel_spmd

    res = run_bass_kernel_spmd(nc, in_maps, core_ids=list(range(NC)))
    if _results_out is not None:
        _results_out.append(res)

    return _postprocess([res.results[c]["out"] for c in range(NC)],
                        prep["assign"])
